# revision 53
# baseline (speedup 1.0000x reference)
"""GQA attention (B=2,S=2048,H=2048, 32 Q heads / 8 KV heads, head_dim 64, RoPE,
full non-causal softmax) on 8 TRN2 NeuronCores.

Sharding: tensor-parallel over KV heads. Core i owns KV head i and Q heads
4i..4i+3 (the GQA group). Each core computes its 4 heads of attention plus the
partial O-projection over its 256 output dims; the 8 partials are summed on the
host (pure unshard of the partial-sum shards).

Device layouts are transposed ("dims on partitions") so every matmul contracts
on the partition axis with zero on-device transposes of activations:
  x.T [2048, 4096]        (host-pretransposed, bf16)
  Q.T [256, 4096] = wqT.T-chunks @ x.T-chunks      (2 SBUF tiles of [128, T])
  K.T/V.T [64, 4096]      (col-tiled into one PSUM bank pair)
  scores.T [keys, q] = K.T-slice.T @ Q.T-slice     (per 128-key tile)
  p.T = exp(scores.T/8)   (ScalarE, PSUM->SBUF bf16, scale fused)
  attn_o.T += V-chunk.T @ p.T-chunk ; rowsums += ones.T @ p.T-chunk
  out[t,:] partial = attn_o.T-chunks.T @ woT-chunks
RoPE rotate_half is a partition swap done with tiny SBUF->SBUF DMAs; the sign
flip is folded into the host-precomputed sin table.
"""

import sys

sys.path.insert(0, "/opt/trn_rl_repo")

import math

import ml_dtypes
import numpy as np

import concourse.bass as bass
import concourse.mybir as mybir
import concourse.tile as tile
from concourse import bacc
from concourse.bass_utils import run_bass_kernel_spmd
from concourse.masks import make_identity

BF16 = mybir.dt.bfloat16
F32 = mybir.dt.float32

HIDDEN = 2048
N_HEADS = 32
N_KV_HEADS = 8
HEAD_DIM = 64
KV_GROUPS = 4
ROPE_THETA = 10000.0
BSZ, SEQ = 2, 2048
T = BSZ * SEQ  # 4096 tokens
HCH = HIDDEN // 128  # 16 hidden chunks
NB = T // 512  # 8 token blocks for projections
KT = SEQ // 128  # 16 key tiles per batch
QBLKS = SEQ // 512  # 4 q blocks of 512 per batch
NCORES = 8


def build_nc(trace_scopes: bool = False):
    nc = bacc.Bacc(None, target_bir_lowering=False, debug=False)

    xT = nc.declare_dram_parameter("xT", [NB // 2, 128, HCH, 1024], BF16, isOutput=False)
    wq = nc.declare_dram_parameter("wq", [HIDDEN, 256], BF16, isOutput=False)
    wkv = nc.declare_dram_parameter("wkv", [HIDDEN, 128], BF16, isOutput=False)
    wo = nc.declare_dram_parameter("wo", [256, HIDDEN], BF16, isOutput=False)
    cosT = nc.declare_dram_parameter("cosT", [128, T], BF16, isOutput=False)
    sinT = nc.declare_dram_parameter("sinT", [128, T], BF16, isOutput=False)
    out = nc.declare_dram_parameter("out", [T, HIDDEN], BF16, isOutput=True)

    with tile.TileContext(nc) as tc:
        _build_body(nc, tc, xT, wq, wkv, wo, cosT, sinT, out)
    nc.compile()
    return nc


def _build_body(nc, tc, xT, wq, wkv, wo, cosT, sinT, out):
    with tc.tile_pool(name="persist", bufs=1) as persist:
        _build_inner(nc, tc, persist, xT, wq, wkv, wo, cosT, sinT, out)


def _build_inner(nc, tc, persist, xT, wq, wkv, wo, cosT, sinT, out):
    # ---------------- persistent SBUF tensors ----------------
    wq_s = persist.tile([128, HCH, 256], BF16, name="wq_s")
    wkv_s = persist.tile([128, HCH, 128], BF16, name="wkv_s")
    wo_s0 = persist.tile([128, HIDDEN], BF16, name="wo_s0")
    wo_s1 = persist.tile([128, HIDDEN], BF16, name="wo_s1")
    cos_s = persist.tile([128, T], BF16, name="cos_s")
    sin_s = persist.tile([128, T], BF16, name="sin_s")
    qt0 = persist.tile([128, T], BF16, name="qt0")  # heads 0(p0-63),1(p64-127)
    qt1 = persist.tile([128, T], BF16, name="qt1")  # heads 2,3
    kvt = persist.tile([128, T], BF16, name="kvt")  # K.T rows 0-63, V.T rows 64-127
    k2t = persist.tile([128, T], BF16, name="k2t")  # K.T duplicated on both halves
    # [ones | V | ones]: even-head lhsT = [:,kc,64:192] = [V|ones],
    # odd-head lhsT = [:,kc,0:128] = [ones|V]
    v_s = persist.tile([128, 2 * KT, 192], BF16, name="v_s")
    ident = persist.tile([128, 64], BF16, name="ident")
    ao0 = persist.tile([128, T], BF16, name="ao0")  # attn_o.T heads 0,1
    ao1 = persist.tile([128, T], BF16, name="ao1")  # attn_o.T heads 2,3

    # weight loads (SWDGE - off the SP HWDGE ring used for bulk x loads);
    # wq/wkv first: the first projection matmuls wait on them
    for c in range(HCH):
        nc.gpsimd.dma_start(wq_s[:, c, :], wq[c * 128 : (c + 1) * 128, :])
        nc.gpsimd.dma_start(wkv_s[:, c, :], wkv[c * 128 : (c + 1) * 128, :])
    nc.gpsimd.dma_start(cos_s[:], cosT[:])
    nc.gpsimd.dma_start(sin_s[:], sinT[:])
    nc.gpsimd.dma_start(wo_s0[:], wo[0:128, :])
    nc.gpsimd.dma_start(wo_s1[:], wo[128:256, :])
    nc.gpsimd.memset(v_s.rearrange("p c m -> p (c m)")[:, :], 1.0)
    make_identity(nc, ident[0:64, :])
    make_identity(nc, ident[64:128, :])

    def rope_q(qt, sq_pool, ts0, width):
        sl_t = slice(ts0, ts0 + width)
        sq = sq_pool.tile([128, 1024], BF16, name="sq")
        sq = sq[:, 0:width]
        nc.vector.tensor_copy(sq[0:32, :], qt[32:64, sl_t])
        nc.vector.tensor_copy(sq[32:64, :], qt[0:32, sl_t])
        nc.vector.tensor_copy(sq[64:96, :], qt[96:128, sl_t])
        nc.vector.tensor_copy(sq[96:128, :], qt[64:96, sl_t])
        nc.vector.tensor_tensor(sq[:], sq[:], sin_s[:, sl_t], mybir.AluOpType.mult)
        nc.vector.tensor_tensor(
            qt[:, sl_t], qt[:, sl_t], cos_s[:, sl_t], mybir.AluOpType.mult
        )
        nc.vector.tensor_tensor(qt[:, sl_t], qt[:, sl_t], sq[:], mybir.AluOpType.add)

    junk = persist.tile([128, 128], BF16, name="junk")
    nc.vector.memset(junk[:], 0.0)
    # preload the exp LUT set (~2.7us table load + drain) while ScalarE is
    # otherwise idle, instead of paying it on the first real softmax tile
    nc.scalar.activation(
        junk[:, 64:65], junk[:, 0:1], mybir.ActivationFunctionType.Exp
    )

    # ---------------- phase B: QKV projections + RoPE (np_-sliced) ----------
    with (
        tc.tile_pool(name="xs_pool", bufs=2) as xs_pool,
        tc.tile_pool(name="proj_psum", bufs=1, space="PSUM") as proj_psum,
        tc.tile_pool(name="tr_psum", bufs=2, space="PSUM") as tr_psum,
        tc.tile_pool(name="rope_pool", bufs=2) as rope_pool,
    ):
        # HAM pre-warm: keep the PE busy during the initial DMA head so the
        # clock gate opens (1.2 -> 2.4 GHz) before the real matmuls start.
        warm = tr_psum.tile([128, 128], F32, name="warm", tag="pst")
        for _ in range(36):
            nc.tensor.matmul(warm[:], junk[:], junk[:])
        pending_tr = []
        for np_ in range(NB // 2):
            ts0 = np_ * 1024
            sl_t = slice(ts0, ts0 + 1024)
            psq0 = proj_psum.tile([128, 1024], F32, name="psq0")
            psq1 = proj_psum.tile([128, 1024], F32, name="psq1")
            pskv = proj_psum.tile([128, 1024], F32, name="pskv")
            xs = xs_pool.tile([128, HCH, 1024], BF16, name="xs")
            if np_ == 0:
                # split the first load so the PE can start after ~256KB
                for c_ in range(HCH):
                    eng = nc.sync if c_ % 2 == 0 else nc.scalar
                    eng.dma_start(xs[:, c_, :], xT[np_, :, c_, :])
            else:
                # two HWDGE rings (SP + ACT) in parallel; ScalarE is idle here
                nc.sync.dma_start(xs[:, 0:HCH:2, :], xT[np_, :, 0:HCH:2, :])
                nc.scalar.dma_start(xs[:, 1:HCH:2, :], xT[np_, :, 1:HCH:2, :])
            for c in range(HCH):
                st = dict(start=(c == 0), stop=(c == HCH - 1))
                for h in range(2):
                    sl = slice(h * 512, (h + 1) * 512)
                    nc.tensor.matmul(psq0[:, sl], wq_s[:, c, 0:128], xs[:, c, sl], **st)
                    nc.tensor.matmul(psq1[:, sl], wq_s[:, c, 128:256], xs[:, c, sl], **st)
                    nc.tensor.matmul(pskv[0:64, sl], wkv_s[:, c, 0:64], xs[:, c, sl], **st)
                    nc.tensor.matmul(pskv[64:128, sl], wkv_s[:, c, 64:128], xs[:, c, sl], **st)
            nc.vector.tensor_copy(qt0[:, sl_t], psq0[:])
            nc.vector.tensor_copy(qt1[:, sl_t], psq1[:])
            nc.vector.tensor_copy(kvt[:, sl_t], pskv[:])
            # RoPE on this slice
            rope_q(qt0, rope_pool, ts0, 1024)
            rope_q(qt1, rope_pool, ts0, 1024)
            sk = rope_pool.tile([64, 1024], BF16, name="sk")
            nc.vector.tensor_copy(sk[0:32, :], kvt[32:64, sl_t])
            nc.vector.tensor_copy(sk[32:64, :], kvt[0:32, sl_t])
            nc.vector.tensor_tensor(
                sk[:], sk[:], sin_s[0:64, sl_t], mybir.AluOpType.mult
            )
            kr_ = rope_pool.tile([64, 1024], BF16, name="kr_")
            nc.vector.tensor_tensor(
                kr_[:], kvt[0:64, sl_t], cos_s[0:64, sl_t], mybir.AluOpType.mult
            )
            nc.vector.tensor_tensor(
                k2t[0:64, sl_t], kr_[:], sk[:], mybir.AluOpType.add
            )
            nc.vector.tensor_copy(k2t[64:128, sl_t], k2t[0:64, sl_t])

            # V transpose for this slice, deferred one np_ so the PE is not
            # head-of-line blocked waiting on the kvt evacuation
            def make_tr(np_c):
                def go():
                    for c8 in range(8):
                        c = np_c * 8 + c8
                        s0 = np_c * 1024 + c8 * 128
                        pst = tr_psum.tile([128, 64], BF16, name="pst")
                        nc.tensor.transpose(
                            pst[:], kvt[64:128, s0 : s0 + 128], ident[64:128, :]
                        )
                        nc.vector.tensor_copy(v_s[:, c, 64:128], pst[:])
                return go
            pending_tr.append(make_tr(np_))
            if np_ >= 1:
                pending_tr.pop(0)()
        for go in pending_tr:
            go()
        # keep the PE warm through the RoPE tail into the attention phase
        warm2 = tr_psum.tile([128, 128], F32, name="warm2", tag="pst")
        for _ in range(32):
            nc.tensor.matmul(warm2[:], junk[:], junk[:])

    # ---------------- phase D: attention + O-projection ----------------
    # O-projection work for block (b,qb) is emitted interleaved into the NEXT
    # block's kt loop so the PE never runs a long oproj burst that starves the
    # exp pipeline on ScalarE.
    blocks = [(b, qb) for b in range(BSZ) for qb in range(QBLKS)]
    with (
        tc.tile_pool(name="sc_psum", bufs=2, space="PSUM") as sc_psum,
        tc.tile_pool(name="av_psum", bufs=1, space="PSUM") as av_psum,
        tc.tile_pool(name="op_psum", bufs=2, space="PSUM") as op_psum,
        tc.tile_pool(name="pt_pool", bufs=7) as pt_pool,
        tc.tile_pool(name="rrs_pool", bufs=3) as rrs_pool,
        tc.tile_pool(name="ost_pool", bufs=3) as ost_pool,
    ):
        def emit_oproj(qs, on_scalar=False):
            state = {}

            def unit(tb, oj):
                def go():
                    ts0 = qs + tb * 128
                    if oj == 0:
                        state[tb] = ost_pool.tile([128, HIDDEN], BF16, name="ost")
                    ost = state[tb]
                    pop = op_psum.tile([128, 512], F32, name="pop")
                    nc.tensor.matmul(
                        pop[:],
                        ao0[:, ts0 : ts0 + 128],
                        wo_s0[:, oj * 512 : (oj + 1) * 512],
                        start=True,
                        stop=False,
                    )
                    nc.tensor.matmul(
                        pop[:],
                        ao1[:, ts0 : ts0 + 128],
                        wo_s1[:, oj * 512 : (oj + 1) * 512],
                        start=False,
                        stop=True,
                    )
                    if on_scalar:
                        nc.scalar.copy(ost[:, oj * 512 : (oj + 1) * 512], pop[:])
                    else:
                        nc.vector.tensor_copy(ost[:, oj * 512 : (oj + 1) * 512], pop[:])
                    if oj == 3:
                        nc.sync.dma_start(out[ts0 : ts0 + 128, :], ost[:])

                return go

            return [unit(tb, oj) for tb in range(4) for oj in range(4)]

        pending = []
        for bi, (b, qb) in enumerate(blocks):
            qs = b * SEQ + qb * 512
            for hp, (qt, ao) in enumerate(((qt0, ao0), (qt1, ao1))):
                pse = av_psum.tile([128, 512], F32, name="pse")
                pso = av_psum.tile([128, 512], F32, name="pso")
                SKEW = 4
                pts = {}

                def av(kt):
                    kc = b * KT + kt
                    st = dict(start=(kt == 0), stop=(kt == KT - 1))
                    pt = pts.pop(kt)
                    nc.tensor.matmul(pse[:], v_s[:, kc, 64:192], pt[:, 0:512], **st)
                    nc.tensor.matmul(pso[:], v_s[:, kc, 0:128], pt[:, 512:1024], **st)

                for kt in range(KT):
                    kr = b * SEQ + kt * 128
                    psa = sc_psum.tile([128, 1024], F32, name="psa")
                    nc.tensor.matmul(
                        psa[:, 0:512],
                        k2t[0:64, kr : kr + 128],
                        qt[0:64, qs : qs + 512],
                    )
                    nc.tensor.matmul(
                        psa[:, 512:1024],
                        k2t[64:128, kr : kr + 128],
                        qt[64:128, qs : qs + 512],
                    )
                    pt = pt_pool.tile([128, 1024], BF16, name="pt")
                    nc.scalar.activation(
                        pt[:],
                        psa[:],
                        mybir.ActivationFunctionType.Exp,
                        scale=1.0 / math.sqrt(HEAD_DIM),
                    )
                    pts[kt] = pt
                    if kt >= SKEW:
                        av(kt - SKEW)
                    if kt % 2 == 0 and pending:
                        pending.pop(0)()
                for kt in range(KT - SKEW, KT):
                    av(kt)
                # rowsums: partition-shifted copies straight from PSUM
                rrs = rrs_pool.tile([128, 512], F32, name="rrs")
                nc.vector.tensor_copy(rrs[0:64, :], pse[64:128, :])
                nc.vector.tensor_copy(rrs[64:128, :], pso[0:64, :])
                rri = rrs_pool.tile([128, 512], F32, name="rri")
                nc.vector.reciprocal_approx_fast(rri[:], rrs[:])
                nc.vector.tensor_tensor(
                    ao[0:64, qs : qs + 512],
                    pse[0:64, :],
                    rri[0:64, :],
                    mybir.AluOpType.mult,
                )
                nc.vector.tensor_tensor(
                    ao[64:128, qs : qs + 512],
                    pso[64:128, :],
                    rri[64:128, :],
                    mybir.AluOpType.mult,
                )
            for go in pending:
                go()
            pending = emit_oproj(qs, on_scalar=(bi == len(blocks) - 1))
        for go in pending:
            go()


def _host_prep(hidden_states, position_ids, Wq, Wk, Wv, Wo):
    bf = ml_dtypes.bfloat16
    x = np.ascontiguousarray(hidden_states.reshape(T, HIDDEN))
    xT = x.T.astype(bf)  # [HIDDEN, T]
    # block to [NB, HCH, 128, 512] so each projection tile is one contiguous read
    xTb = np.ascontiguousarray(
        xT.reshape(HCH, 128, NB // 2, 1024).transpose(2, 1, 0, 3)
    )

    # RoPE tables, transposed to [64, T], sign of sin folded for rotate_half,
    # then stacked twice to cover two heads per SBUF tile.
    inv_freq = 1.0 / (
        ROPE_THETA ** (np.arange(0, HEAD_DIM, 2, dtype=np.float32) / HEAD_DIM)
    )
    pos = position_ids.astype(np.float32).reshape(BSZ, SEQ)
    freqs = pos[:, :, None] * inv_freq[None, None, :]  # [B, S, 32]
    emb = np.concatenate([freqs, freqs], axis=-1)  # [B, S, 64]
    cos = np.cos(emb).reshape(T, HEAD_DIM).T  # [64, T]
    sin = np.sin(emb).reshape(T, HEAD_DIM).T.copy()
    sin[0:32, :] *= -1.0  # rotate_half sign fold
    cosT = np.ascontiguousarray(np.concatenate([cos, cos], axis=0)).astype(bf)
    sinT = np.ascontiguousarray(np.concatenate([sin, sin], axis=0)).astype(bf)

    in_maps = []
    for c in range(NCORES):
        q0 = c * KV_GROUPS * HEAD_DIM  # 256*c
        wq_c = np.ascontiguousarray(Wq[q0 : q0 + 256, :].T).astype(bf)  # [2048, 256]
        wk_c = Wk[c * 64 : (c + 1) * 64, :].T  # [2048, 64]
        wv_c = Wv[c * 64 : (c + 1) * 64, :].T
        wkv_c = np.ascontiguousarray(np.concatenate([wk_c, wv_c], axis=1)).astype(bf)
        wo_c = np.ascontiguousarray(Wo[:, q0 : q0 + 256].T).astype(bf)  # [256, 2048]
        in_maps.append(
            {
                "xT": xTb,
                "wq": wq_c,
                "wkv": wkv_c,
                "wo": wo_c,
                "cosT": cosT,
                "sinT": sinT,
            }
        )
    return in_maps


_RUN_KW = {}


def kernel(hidden_states, position_ids, Wq, Wk, Wv, Wo):
    # accept numpy or jax arrays
    hidden_states = np.asarray(hidden_states, dtype=np.float32)
    position_ids = np.asarray(position_ids)
    Wq, Wk, Wv, Wo = (np.asarray(w, dtype=np.float32) for w in (Wq, Wk, Wv, Wo))
    in_maps = _host_prep(hidden_states, position_ids, Wq, Wk, Wv, Wo)
    nc = build_nc()
    res = run_bass_kernel_spmd(nc, in_maps, core_ids=list(range(NCORES)), **_RUN_KW)
    acc = np.zeros((T, HIDDEN), dtype=np.float32)
    for i in range(NCORES):
        acc += res.results[i]["out"].astype(np.float32)
    if _RUN_KW.get("trace"):
        kernel.last_exec_time_ns = res.exec_time_ns
        kernel.last_result = res
    return acc.reshape(BSZ, SEQ, HIDDEN)
